# revision 46
# baseline (speedup 1.0000x reference)
"""Multi-head causal attention on 8 TRN2 NeuronCores — one head per core.

Full inputs in, full output out. Per core (head h):
  Q^T/K^T = W^T x^T   (PE bf16, duplicated into both partition halves)
  S^T[j,i] = K_j . Q_i  (PE bf16, 128-deep duplicated contraction — keeps
                         the PE activity monitor at 2.4 GHz; the doubled
                         product folds into the exp scale)
  P^T = exp(S^T/16)     (ScalarE, 1024-wide calls, double-buffered PSUM)
  O^T[v,i] accum += V'[j,(v|1)]^T P^T[j,i]  (PE bf16; row 64 = sumexp)
  out[i,o] = (O^T[:,i]/sumexp_i)^T W_o      (PE + DVE row scale on evac)
Host sums the 8 per-head partial outputs (bf16 partials, f32 sum).
"""

import numpy as np
import ml_dtypes

import concourse.bass as bass
import concourse.mybir as mybir
import concourse.tile as tile
from concourse import bacc
from concourse.bass_utils import run_bass_kernel_spmd

BF16 = mybir.dt.bfloat16
F32 = mybir.dt.float32

S = 4096
D_IN = 512
D_K = 64
D_V = 64
D_OUT = 512
H = 8
NJT = S // 128   # 32 key tiles
NCH = S // 512   # 8 query chunks
NCK = D_IN // 128  # 4 contraction chunks for projections

_CACHE = {}

# Finalize chunk halves before their PSUM accumulation group formally stops
# (safe on HW: the read columns receive no further writes). CoreSim rejects
# mid-group reads, so sim validation sets this False.
EARLY_FIN = True


def _emit(nc, tc, ctx_pools):
    import contextlib

    xT_d = nc.dram_tensor("xT", [D_IN, S], BF16, kind="ExternalInput").ap()
    wq_d = nc.dram_tensor("wq", [D_IN, 128], BF16, kind="ExternalInput").ap()
    wk_d = nc.dram_tensor("wk", [D_IN, 128], BF16, kind="ExternalInput").ap()
    wv_d = nc.dram_tensor("wv", [D_IN, D_V], BF16, kind="ExternalInput").ap()
    wo_d = nc.dram_tensor("wo", [D_V, D_OUT], BF16, kind="ExternalInput").ap()
    mask_d = nc.dram_tensor("mask", [128, 128], BF16, kind="ExternalInput").ap()
    ident_d = nc.dram_tensor("ident", [128, 128], BF16, kind="ExternalInput").ap()
    out_d = nc.dram_tensor("out", [S, D_OUT], BF16, kind="ExternalOutput").ap()

    Exp = mybir.ActivationFunctionType.Exp

    with contextlib.ExitStack() as ctx:
        const = ctx.enter_context(tc.tile_pool(name="const", bufs=1))
        persist = ctx.enter_context(tc.tile_pool(name="persist", bufs=1))
        small = ctx.enter_context(tc.tile_pool(name="small", bufs=3))
        outp = ctx.enter_context(tc.tile_pool(name="outp", bufs=3))

        # ---- constants ----
        # wq/wk arrive column-duplicated [512, 128] so the projection fills
        # both partition halves of Q^T/K^T (128-deep score contraction)
        wq_sb = const.tile([128, NCK * 128], BF16)
        wk_sb = const.tile([128, NCK * 128], BF16)
        wv_sb = const.tile([128, NCK * D_V], BF16)
        wo_sb = const.tile([D_V, D_OUT], BF16)
        mask_sb = const.tile([128, 128], BF16)
        ident_sb = const.tile([128, 128], BF16)

        # persistent activations
        qt = persist.tile([128, S], BF16)   # Q^T duplicated in both halves
        kt = persist.tile([128, S], BF16)   # K^T duplicated in both halves
        vp = persist.tile([128, NJT * 65], BF16)  # V' tiles [128, 65] per jt

        # ones column of every V' tile: strided [128, NJT] memset
        nc.vector.memset(
            vp.rearrange("p (j w) -> p j w", w=65)[:, :, 64], 1.0)

        # scratch used to warm up the PE clock gate / ScalarE act table
        # while the x^T DMAs are still in flight
        zt = const.tile([128, 256], BF16)
        nc.vector.memset(zt, 0.0)

        # ---- stage A: x^T spread across all four DMA queues, st-major, so
        # the first 512-col block lands ~0.7us after kickoff and the full
        # 4 MB is resident by ~7us (vs 21us on one queue) ----
        pt_pool = ctx.enter_context(tc.tile_pool(name="pt", bufs=1))
        xtp_ctx = contextlib.ExitStack()
        xtp = xtp_ctx.enter_context(tc.tile_pool(name="xt", bufs=1))
        xts = [xtp.tile([128, S], BF16, tag=f"xt{c}", name=f"xt{c}")
               for c in range(NCK)]

        def wload(q, sb, dram, wid):
            # whole weight in one DMA: SBUF [128, c*wid] <- DRAM [(c p), wid]
            q.dma_start(
                out=sb.rearrange("p (c j) -> p c j", c=NCK),
                in_=dram.rearrange("(c p) j -> p c j", p=128),
            )

        def xdma(q, c, st, w=1):
            sl = slice(st * 512, (st + w) * 512)
            q.dma_start(out=xts[c][:, sl], in_=xT_d[c * 128:(c + 1) * 128, sl])

        # 3 DMA queues: sync + scalar are hardware DGE (~0.6us issue,
        # ~1.5us completion-sem latency); gpsimd is software DGE (slower
        # completions) and gets the late bulk. DMA issue cost is ~fixed
        # per dma_start, so st>=2 ships as 1024/2048-col blocks. scalar's
        # queue must free up by ~11us for the exp stream.
        # x singles, st-major, spread over the 3 queues in need order.
        # Aggregate HBM read BW (~330 GB/s) bounds total arrival; what
        # matters is that completion ORDER tracks consumption order.
        wload(nc.sync, wq_sb, wq_d, 128)
        wload(nc.gpsimd, wk_sb, wk_d, 128)
        wload(nc.gpsimd, wv_sb, wv_d, D_V)
        xdma(nc.scalar, 3, 0)
        xdma(nc.scalar, 3, 1)
        for st in range(6):
            xdma(nc.sync, 0, st)
            xdma(nc.sync, 2, st)
            if st >= 2:
                xdma(nc.scalar, 3, st)
            xdma(nc.gpsimd, 1, st)
            if st == 3:
                nc.sync.dma_start(out=mask_sb, in_=mask_d)
            if st < 2:
                xdma(nc.gpsimd, 3, st + 6)
        xdma(nc.sync, 0, 6)
        xdma(nc.gpsimd, 2, 6)
        xdma(nc.gpsimd, 1, 6)
        xdma(nc.sync, 2, 7)
        xdma(nc.gpsimd, 1, 7)
        xdma(nc.sync, 0, 7)
        nc.gpsimd.dma_start(out=wo_sb, in_=wo_d)
        nc.gpsimd.dma_start(out=ident_sb, in_=ident_d)

        evn = [0]

        def evac_engine():
            # PSUM can only be read by PE/DVE/ScalarE (GpSimd is barred),
            # so evacuations stay on DVE
            evn[0] += 1
            return nc.vector

        def proj(w_sb, wid, dest, st, pool, tag="psA", alt=False,
                 corder=(0, 1, 2, 3)):
            sl = bass.ts(st, 512)
            ps = pool.tile([wid, 512], F32, tag=tag, name=f"ps{wid}_{st}")
            for i, c in enumerate(corder):
                nc.tensor.matmul(
                    ps,
                    lhsT=w_sb[:, c * wid:(c + 1) * wid],
                    rhs=xts[c][:, sl],
                    start=(i == 0),
                    stop=(i == NCK - 1),
                )
            (evac_engine() if alt else nc.vector).tensor_copy(dest[:, sl], ps)

        with tc.tile_pool(name="psA", bufs=4, space="PSUM") as psA:
            # short HAM warm-up burst: ramps the PE clock while the first
            # x^T blocks land; real projections continue the busy run
            hps = psA.tile([128, 256], F32, tag="heat")
            for _ in range(8):
                nc.tensor.matmul(hps, lhsT=zt[:, 0:128], rhs=zt,
                                 start=True, stop=True)
            # c-accumulation order matches x-tile arrival order (c3 on
            # scalar lands first, then c0/c2 on sync, then c1 on gpsimd)
            proj(wq_sb, 128, qt, 0, psA, corder=(3, 0, 2, 1))
            proj(wk_sb, 128, kt, 0, psA, corder=(3, 0, 2, 1))
            proj(wq_sb, 128, qt, 1, psA, corder=(3, 0, 2, 1))


        # ---- fused pass: S^T+exp, with O^T bursts filling PE exp-wait gaps ----
        from collections import deque
        pts = []
        pending = deque()  # closures, each emits one PE-side step of pass 2
        state = {"done": 0}

        def groups(jt):
            # 1024-col exp groups; jt=0 leads with a 512 so the first exp
            # only needs Q st0 (not st1)
            i0 = jt * 128
            if jt == 0:
                bounds = [0, 512, 1536, 2560, 3584, 4096]
            else:
                bounds = list(range(i0, S, 1024)) + [S]
            return list(zip(bounds[:-1], bounds[1:]))

        TOT_GROUPS = sum(len(groups(jt)) for jt in range(NJT))

        def drain(n):
            for _ in range(n):
                if not pending:
                    return
                pending.popleft()()
                state["done"] += 1

        def vp_direct(jt):
            # V' tile [128 keys, 64] = x_tile^T @ W_v — no transpose needed
            def go():
                pv = psFil.tile([128, 64], F32, tag="bank", name=f"pv{jt}")
                for c in range(NCK):
                    nc.tensor.matmul(
                        pv,
                        lhsT=xts[c][:, jt * 128:(jt + 1) * 128],
                        rhs=wv_sb[:, c * D_V:(c + 1) * D_V],
                        start=(c == 0),
                        stop=(c == NCK - 1),
                    )
                nc.vector.tensor_copy(vp[:, jt * 65:jt * 65 + 64], pv)
            return go

        def filler_proj(w_sb, wid, dest, st):
            def go():
                proj(w_sb, wid, dest, st, psFil, tag="bank", alt=True)
            return go

        accs = {}
        nq = [0] * NCH
        findex = [0] * NCH  # finalize-piece counter per chunk (DMA queue mix)

        def top_up(c, hi):
            hi = min(hi, 4 * c + 4)
            if hi > nq[c]:
                enqueue_ot(c, range(nq[c], hi))
                nq[c] = hi

        def enqueue_ot(c, j2s):
            if c not in accs:
                accs[c] = psOt.tile([65, 512], F32, tag="acc",
                                    name=f"acc{c}")
            acc = accs[c]
            jt_last = 4 * c + 3

            def ot_mm(j2):
                def go():
                    lo = max(c * 512, j2 * 128)
                    hi = (c + 1) * 512
                    nc.tensor.matmul(
                        acc[:, lo - c * 512:hi - c * 512],
                        lhsT=vp[:, j2 * 65:(j2 + 1) * 65],
                        rhs=pts[j2][:, lo - j2 * 128:hi - j2 * 128],
                        start=(j2 == 0),
                        stop=(j2 == jt_last),
                    )
                return go

            for j2 in j2s:
                pending.append(ot_mm(j2))

        def enqueue_fin(c, lo=0, hi=512, last=False):
            # finalize output rows [c*512+lo, c*512+hi); `last` routes work
            # onto ScalarE / the freed accumulator banks after exp is done,
            # and uses a PE transpose for sumexp instead of a DMA (shorter
            # latency chain on the tail)
            acc = accs[c]
            ibs = list(range(lo // 128, hi // 128))

            def evac():
                ot_bf = small.tile([65, hi - lo], BF16, tag="otbf",
                                   name=f"otbf{c}_{lo}")
                if last:
                    nc.scalar.copy(ot_bf, acc[:, lo:hi])
                else:
                    nc.vector.tensor_copy(ot_bf, acc[:, lo:hi])
                rcols = small.tile([128, 4], F32, tag="rcols",
                                   name=f"rc{c}_{lo}")
                nb = len(ibs)
                if last:
                    # PE transpose of each [65,128] block; sumexp lands in
                    # column 64 of each 68-wide slot (4-byte aligned)
                    tp = psOt.tile([128, 68 * nb], BF16, tag="acc",
                                   name=f"tp{c}_{lo}")

                    def tp_mm():
                        for k in range(nb):
                            nc.tensor.transpose(
                                tp[:, k * 68:k * 68 + 65],
                                ot_bf[:, k * 128:(k + 1) * 128],
                                ident_sb[0:65, 0:65],
                            )
                    pending.append(tp_mm)

                    def recip():
                        nc.vector.reciprocal(
                            rcols[:, 0:nb],
                            tp.rearrange("p (b w) -> p b w", w=68)[:, 0:nb, 64])
                    pending.append(recip)
                else:
                    # sumexp row -> per-partition columns: tiny transposing
                    # DMAs spread across queues so they run in parallel
                    se_bf = small.tile([128, 4], BF16, tag="se_bf",
                                       name=f"se{c}_{lo}")
                    for k, ib in enumerate(ibs):
                        q = (nc.sync, nc.gpsimd)[
                            0 if c >= NCH - 2 else (findex[c] + k) % 2]
                        q.dma_start(
                            out=se_bf[:, k:k + 1],
                            in_=ot_bf[64:65, ib * 128 - lo:(ib + 1) * 128 - lo],
                        )

                    def recip():
                        nc.vector.reciprocal(rcols[:, 0:nb], se_bf[:, 0:nb])
                    pending.append(recip)
                pos = {}

                def po_mm(k, ib):
                    # independent of rcols — keeps the PE busy while the
                    # sumexp reciprocal chain resolves. The final part uses
                    # the accumulator banks (free by then) instead of psFil.
                    def go():
                        pool, tg = (psOt, "acc") if last else (psFil, "bank")
                        po = pool.tile([128, 512], F32, tag=tg,
                                       name=f"po{c}_{ib}")
                        pos[k] = po
                        nc.tensor.matmul(
                            po,
                            lhsT=ot_bf[0:64, ib * 128 - lo:(ib + 1) * 128 - lo],
                            rhs=wo_sb,
                            start=True,
                            stop=True,
                        )
                    return go

                def po_scale(k, ib):
                    def go():
                        ob = outp.tile([128, 512], BF16, tag="ob")
                        if last and k == 0:
                            nc.scalar.mul(ob, pos[k], rcols[:, k:k + 1])
                        elif last:
                            nc.vector.tensor_scalar_mul(
                                ob, pos[k], rcols[:, k:k + 1])
                        else:
                            evac_engine().tensor_scalar_mul(
                                ob, pos[k], rcols[:, k:k + 1])
                        oq = (nc.sync, nc.gpsimd)[
                            0 if c >= NCH - 2 else (findex[c] + k) % 2]
                        oq.dma_start(
                            out=out_d[c * 512 + ib * 128:
                                      c * 512 + (ib + 1) * 128, :],
                            in_=ob,
                        )
                    return go

                prev = None
                for k, ib in enumerate(ibs):
                    pending.append(po_mm(k, ib))
                    if prev is not None:
                        pending.append(po_scale(*prev))
                    prev = (k, ib)
                pending.append(po_scale(*prev))

            findex[c] += 1
            pending.append(evac)

        with tc.tile_pool(name="psB", bufs=2, space="PSUM") as psB, \
             tc.tile_pool(name="psFil", bufs=2, space="PSUM") as psFil, \
             tc.tile_pool(name="psOt", bufs=2, space="PSUM") as psOt:
            # filler work: remaining Q/K projections, V projections + V'
            # tiles. EMISSION ORDER IS LOGICAL ORDER in the tile framework:
            # jt0's score group over cols needing Q st_k must come after the
            # Q st_k filler (uniform 2-per-group drain satisfies this).
            for st in range(2, NCH):
                pending.append(filler_proj(wq_sb, 128, qt, st))
            for st in range(1, NCH):
                pending.append(filler_proj(wk_sb, 128, kt, st))
            for j2 in range(NJT):
                pending.append(vp_direct(j2))
            N_HEAD = len(pending)  # closures that still read the x^T tiles
            gdone = 0
            pt_blk, blk_off = None, 0
            for jt in range(NJT):
                i0 = jt * 128           # diagonal start
                # P^T tiles live in 4 merged block allocations (8 jt each):
                # pool-slot teardown costs ~4 semaphore ops per slot per
                # engine at kernel exit, so fewer slots = shorter epilogue
                if jt % 8 == 0:
                    blk_w = sum(S - 128 * j for j in range(jt, jt + 8))
                    pt_blk = pt_pool.tile([128, blk_w], BF16,
                                          tag=f"ptb{jt // 8}")
                    blk_off = 0
                pt = pt_blk[:, blk_off:blk_off + (S - i0)]
                blk_off += S - i0
                pts.append(pt)
                ktile = kt[:, jt * 128:(jt + 1) * 128]
                for gi, (g0, ghi) in enumerate(groups(jt)):
                    w = ghi - g0
                    ps = psB.tile([128, 1024], F32, tag="psB")
                    for lo in range(g0, ghi, 512):
                        hi = min(lo + 512, ghi)
                        nc.tensor.matmul(
                            ps[:, lo - g0:hi - g0],
                            lhsT=ktile,
                            rhs=qt[:, lo:hi],
                            start=True,
                            stop=True,
                        )
                    nc.scalar.activation(
                        pt[:, g0 - i0:ghi - i0],
                        ps[:, 0:w],
                        Exp,
                        scale=0.0625,  # 1/sqrt(64) / 2 (duplicated contraction)
                    )
                    gdone += 1
                    # pull PE filler work at a rate that empties the
                    # queue just as the exp stream ends
                    left = max(1, TOT_GROUPS - 6 - gdone)
                    drain(max(2, -(-len(pending) // left)))
                # causal mask on the diagonal 128x128 block (SBUF->SBUF, so
                # it can run on GpSimd — keeps the DVE queue short for the
                # PSUM evacuations that gate psFil slots)
                nc.vector.tensor_mul(pt[:, 0:128], pt[:, 0:128], mask_sb)
                c = jt // 4
                if jt % 4 == 1:
                    top_up(c, jt + 1)
                    if jt == NJT - 3 and EARLY_FIN:
                        # chunk 7 rows 3584-3840 are complete — finalize
                        # them under the remaining exp stream
                        enqueue_fin(c, 0, 256, last=False)
                elif jt % 4 == 2:
                    # rows [c*512, c*512+256) only need j2 <= 4c+1: finalize
                    # the first half early to spread evac/Wo/DMA load.
                    # (Reads the acc mid-accumulation-group on untouched
                    # columns — fine on HW, rejected by CoreSim; EARLY_FIN
                    # is turned off for sim validation runs.)
                    top_up(c, jt + 1)
                    if EARLY_FIN:
                        if jt == NJT - 2:
                            enqueue_fin(c, 256, 384, last=False)
                        else:
                            enqueue_fin(c, 0, 256, last=False)
                elif jt % 4 == 3:
                    top_up(c, jt + 1)
                    if c + 1 < NCH:
                        if EARLY_FIN:
                            enqueue_fin(c, 256, 512, last=False)
                        else:
                            enqueue_fin(c, 0, 512, last=False)
                        # pre-enqueue the next chunk's already-available
                        # contributions so its O^T work spreads out early
                        top_up(c + 1, jt + 1)
                    else:
                        if EARLY_FIN:
                            enqueue_fin(c, 384, 512, last=True)
                        else:
                            enqueue_fin(c, 0, 512, last=True)
                if jt == 19:
                    # open the last chunk extra-early (its accumulator takes
                    # the PSUM slot acc4 just vacated) so its O^T matmuls
                    # hide under the remaining exp stream instead of
                    # trailing it
                    top_up(NCH - 1, jt + 1)
                if jt == 9:
                    # finish everything that reads x^T, then free those tiles
                    # before the P^T pool reaches peak size; open chunk 3
                    # early (its slot was vacated back at jt=7) so the
                    # filler queue isn't dry right after this burst
                    drain(max(0, N_HEAD - state["done"]))
                    xtp_ctx.close()
                    top_up(3, jt + 1)
            while pending:
                drain(8)


def _build():
    if "nc" in _CACHE:
        return _CACHE["nc"]
    nc = bacc.Bacc("TRN2", target_bir_lowering=False, debug=False)
    with tile.TileContext(nc) as tc:
        _emit(nc, tc, None)
    nc.compile()
    _CACHE["nc"] = nc
    return nc


def build_in_maps(x, W_q, W_k, W_v, W_o):
    bf = ml_dtypes.bfloat16
    xT = np.ascontiguousarray(x.reshape(S, D_IN).T).astype(bf)
    mask = np.triu(np.ones((128, 128), np.float32)).astype(bf)
    ident = np.eye(128, dtype=np.float32).astype(bf)
    in_maps = []
    for h in range(H):
        wq2 = np.concatenate([W_q[h], W_q[h]], axis=1)  # [512, 128]
        wk2 = np.concatenate([W_k[h], W_k[h]], axis=1)
        in_maps.append({
            "xT": xT,
            "wq": np.ascontiguousarray(wq2).astype(bf),
            "wk": np.ascontiguousarray(wk2).astype(bf),
            "wv": np.ascontiguousarray(W_v[h]).astype(bf),
            "wo": np.ascontiguousarray(W_o[h]).astype(bf),
            "mask": mask,
            "ident": ident,
        })
    return in_maps


def kernel(x, W_q, W_k, W_v, W_o):
    nc = _build()
    in_maps = build_in_maps(x, W_q, W_k, W_v, W_o)
    res = run_bass_kernel_spmd(nc, in_maps, core_ids=list(range(H)))
    out = np.zeros((S, D_OUT), np.float32)
    for h in range(H):
        out += np.asarray(res.results[h]["out"], np.float32)
    return out[None]


# revision 47
# speedup vs baseline: 1.0459x; 1.0459x over previous
"""Multi-head causal attention on 8 TRN2 NeuronCores — one head per core.

Full inputs in, full output out. Per core (head h):
  Q^T/K^T = W^T x^T   (PE bf16, duplicated into both partition halves)
  S^T[j,i] = K_j . Q_i  (PE bf16, 128-deep duplicated contraction — keeps
                         the PE activity monitor at 2.4 GHz; the doubled
                         product folds into the exp scale)
  P^T = exp(S^T/16)     (ScalarE, 1024-wide calls, double-buffered PSUM)
  O^T[v,i] accum += V'[j,(v|1)]^T P^T[j,i]  (PE bf16; row 64 = sumexp)
  out[i,o] = (O^T[:,i]/sumexp_i)^T W_o      (PE + DVE row scale on evac)
Host sums the 8 per-head partial outputs (bf16 partials, f32 sum).
"""

import numpy as np
import ml_dtypes

import concourse.bass as bass
import concourse.mybir as mybir
import concourse.tile as tile
from concourse import bacc
from concourse.bass_utils import run_bass_kernel_spmd

BF16 = mybir.dt.bfloat16
F32 = mybir.dt.float32

S = 4096
D_IN = 512
D_K = 64
D_V = 64
D_OUT = 512
H = 8
NJT = S // 128   # 32 key tiles
NCH = S // 512   # 8 query chunks
NCK = D_IN // 128  # 4 contraction chunks for projections

_CACHE = {}

# Finalize chunk halves before their PSUM accumulation group formally stops
# (safe on HW: the read columns receive no further writes). CoreSim rejects
# mid-group reads, so sim validation sets this False.
EARLY_FIN = True


def _emit(nc, tc, ctx_pools):
    import contextlib

    xT_d = nc.dram_tensor("xT", [D_IN, S], BF16, kind="ExternalInput").ap()
    wq_d = nc.dram_tensor("wq", [D_IN, 128], BF16, kind="ExternalInput").ap()
    wk_d = nc.dram_tensor("wk", [D_IN, 128], BF16, kind="ExternalInput").ap()
    wv_d = nc.dram_tensor("wv", [D_IN, D_V], BF16, kind="ExternalInput").ap()
    wo_d = nc.dram_tensor("wo", [D_V, D_OUT], BF16, kind="ExternalInput").ap()
    mask_d = nc.dram_tensor("mask", [128, 128], BF16, kind="ExternalInput").ap()
    ident_d = nc.dram_tensor("ident", [128, 128], BF16, kind="ExternalInput").ap()
    out_d = nc.dram_tensor("out", [S, D_OUT], BF16, kind="ExternalOutput").ap()

    Exp = mybir.ActivationFunctionType.Exp

    with contextlib.ExitStack() as ctx:
        const = ctx.enter_context(tc.tile_pool(name="const", bufs=1))
        persist = ctx.enter_context(tc.tile_pool(name="persist", bufs=1))
        small = ctx.enter_context(tc.tile_pool(name="small", bufs=3))
        outp = ctx.enter_context(tc.tile_pool(name="outp", bufs=3))

        # ---- constants ----
        # wq/wk arrive column-duplicated [512, 128] so the projection fills
        # both partition halves of Q^T/K^T (128-deep score contraction)
        wq_sb = const.tile([128, NCK * 128], BF16)
        wk_sb = const.tile([128, NCK * 128], BF16)
        wv_sb = const.tile([128, NCK * D_V], BF16)
        wo_sb = const.tile([D_V, D_OUT], BF16)
        mask_sb = const.tile([128, 128], BF16)
        ident_sb = const.tile([128, 128], BF16)

        # persistent activations
        qt = persist.tile([128, S], BF16)   # Q^T duplicated in both halves
        kt = persist.tile([128, S], BF16)   # K^T duplicated in both halves
        vp = persist.tile([128, NJT * 65], BF16)  # V' tiles [128, 65] per jt

        # ones column of every V' tile: strided [128, NJT] memset
        nc.vector.memset(
            vp.rearrange("p (j w) -> p j w", w=65)[:, :, 64], 1.0)

        # scratch used to warm up the PE clock gate / ScalarE act table
        # while the x^T DMAs are still in flight
        zt = const.tile([128, 256], BF16)
        nc.vector.memset(zt, 0.0)

        # ---- stage A: x^T spread across all four DMA queues, st-major, so
        # the first 512-col block lands ~0.7us after kickoff and the full
        # 4 MB is resident by ~7us (vs 21us on one queue) ----
        pt_pool = ctx.enter_context(tc.tile_pool(name="pt", bufs=1))
        xtp_ctx = contextlib.ExitStack()
        xtp = xtp_ctx.enter_context(tc.tile_pool(name="xt", bufs=1))
        xts = [xtp.tile([128, S], BF16, tag=f"xt{c}", name=f"xt{c}")
               for c in range(NCK)]

        def wload(q, sb, dram, wid):
            # whole weight in one DMA: SBUF [128, c*wid] <- DRAM [(c p), wid]
            q.dma_start(
                out=sb.rearrange("p (c j) -> p c j", c=NCK),
                in_=dram.rearrange("(c p) j -> p c j", p=128),
            )

        def xdma(q, c, st, w=1):
            sl = slice(st * 512, (st + w) * 512)
            q.dma_start(out=xts[c][:, sl], in_=xT_d[c * 128:(c + 1) * 128, sl])

        # 3 DMA queues: sync + scalar are hardware DGE (~0.6us issue,
        # ~1.5us completion-sem latency); gpsimd is software DGE (slower
        # completions) and gets the late bulk. DMA issue cost is ~fixed
        # per dma_start, so st>=2 ships as 1024/2048-col blocks. scalar's
        # queue must free up by ~11us for the exp stream.
        # x singles, st-major, spread over the 3 queues in need order.
        # Aggregate HBM read BW (~330 GB/s) bounds total arrival; what
        # matters is that completion ORDER tracks consumption order.
        wload(nc.sync, wq_sb, wq_d, 128)
        wload(nc.gpsimd, wk_sb, wk_d, 128)
        wload(nc.gpsimd, wv_sb, wv_d, D_V)
        xdma(nc.scalar, 3, 0)
        xdma(nc.scalar, 3, 1)
        for st in range(6):
            xdma(nc.sync, 0, st)
            xdma(nc.sync, 2, st)
            if st >= 2:
                xdma(nc.scalar, 3, st)
            xdma(nc.gpsimd, 1, st)
            if st == 3:
                nc.sync.dma_start(out=mask_sb, in_=mask_d)
            if st < 2:
                xdma(nc.gpsimd, 3, st + 6)
        xdma(nc.sync, 0, 6)
        xdma(nc.gpsimd, 2, 6)
        xdma(nc.gpsimd, 1, 6)
        xdma(nc.sync, 2, 7)
        xdma(nc.gpsimd, 1, 7)
        xdma(nc.sync, 0, 7)
        nc.gpsimd.dma_start(out=wo_sb, in_=wo_d)
        nc.gpsimd.dma_start(out=ident_sb, in_=ident_d)

        evn = [0]

        def evac_engine():
            # PSUM can only be read by PE/DVE/ScalarE (GpSimd is barred),
            # so evacuations stay on DVE
            evn[0] += 1
            return nc.vector

        def proj(w_sb, wid, dest, st, pool, tag="psA", alt=False,
                 corder=(0, 1, 2, 3)):
            sl = bass.ts(st, 512)
            ps = pool.tile([wid, 512], F32, tag=tag, name=f"ps{wid}_{st}")
            for i, c in enumerate(corder):
                nc.tensor.matmul(
                    ps,
                    lhsT=w_sb[:, c * wid:(c + 1) * wid],
                    rhs=xts[c][:, sl],
                    start=(i == 0),
                    stop=(i == NCK - 1),
                )
            (evac_engine() if alt else nc.vector).tensor_copy(dest[:, sl], ps)

        with tc.tile_pool(name="psA", bufs=4, space="PSUM") as psA:
            # short HAM warm-up burst: ramps the PE clock while the first
            # x^T blocks land; real projections continue the busy run
            hps = psA.tile([128, 256], F32, tag="heat")

            def heat(n):
                # dummy matmuls bridge DMA-completion waits so the PE stays
                # busy (no pstate re-throttle) while stage-A data lands
                for _ in range(n):
                    nc.tensor.matmul(hps, lhsT=zt[:, 0:128], rhs=zt,
                                     start=True, stop=True)

            heat(11)
            # c-accumulation order matches x-tile arrival order (c3 on
            # scalar lands first, then c0/c2 on sync, then c1 on gpsimd)
            proj(wq_sb, 128, qt, 0, psA, corder=(3, 0, 2, 1))
            heat(4)
            proj(wk_sb, 128, kt, 0, psA, corder=(3, 0, 2, 1))
            heat(4)
            proj(wq_sb, 128, qt, 1, psA, corder=(3, 0, 2, 1))
            heat(3)


        # ---- fused pass: S^T+exp, with O^T bursts filling PE exp-wait gaps ----
        from collections import deque
        pts = []
        pending = deque()  # closures, each emits one PE-side step of pass 2
        state = {"done": 0}

        def groups(jt):
            # 1024-col exp groups; jt=0 leads with a 512 so the first exp
            # only needs Q st0 (not st1)
            i0 = jt * 128
            if jt == 0:
                bounds = [0, 512, 1536, 2560, 3584, 4096]
            else:
                bounds = list(range(i0, S, 1024)) + [S]
            return list(zip(bounds[:-1], bounds[1:]))

        TOT_GROUPS = sum(len(groups(jt)) for jt in range(NJT))

        def drain(n):
            for _ in range(n):
                if not pending:
                    return
                pending.popleft()()
                state["done"] += 1

        def vp_direct(jt):
            # V' tile [128 keys, 64] = x_tile^T @ W_v — no transpose needed
            def go():
                pv = psFil.tile([128, 64], F32, tag="bank", name=f"pv{jt}")
                for c in range(NCK):
                    nc.tensor.matmul(
                        pv,
                        lhsT=xts[c][:, jt * 128:(jt + 1) * 128],
                        rhs=wv_sb[:, c * D_V:(c + 1) * D_V],
                        start=(c == 0),
                        stop=(c == NCK - 1),
                    )
                nc.vector.tensor_copy(vp[:, jt * 65:jt * 65 + 64], pv)
            return go

        def filler_proj(w_sb, wid, dest, st):
            def go():
                proj(w_sb, wid, dest, st, psFil, tag="bank", alt=True)
            return go

        accs = {}
        nq = [0] * NCH
        findex = [0] * NCH  # finalize-piece counter per chunk (DMA queue mix)

        def top_up(c, hi):
            hi = min(hi, 4 * c + 4)
            if hi > nq[c]:
                enqueue_ot(c, range(nq[c], hi))
                nq[c] = hi

        def enqueue_ot(c, j2s):
            if c not in accs:
                accs[c] = psOt.tile([65, 512], F32, tag="acc",
                                    name=f"acc{c}")
            acc = accs[c]
            jt_last = 4 * c + 3

            def ot_mm(j2):
                def go():
                    lo = max(c * 512, j2 * 128)
                    hi = (c + 1) * 512
                    nc.tensor.matmul(
                        acc[:, lo - c * 512:hi - c * 512],
                        lhsT=vp[:, j2 * 65:(j2 + 1) * 65],
                        rhs=pts[j2][:, lo - j2 * 128:hi - j2 * 128],
                        start=(j2 == 0),
                        stop=(j2 == jt_last),
                    )
                return go

            for j2 in j2s:
                pending.append(ot_mm(j2))

        def enqueue_fin(c, lo=0, hi=512, last=False):
            # finalize output rows [c*512+lo, c*512+hi); `last` routes work
            # onto ScalarE / the freed accumulator banks after exp is done,
            # and uses a PE transpose for sumexp instead of a DMA (shorter
            # latency chain on the tail)
            acc = accs[c]
            ibs = list(range(lo // 128, hi // 128))

            def evac():
                ot_bf = small.tile([65, hi - lo], BF16, tag="otbf",
                                   name=f"otbf{c}_{lo}")
                if last:
                    nc.scalar.copy(ot_bf, acc[:, lo:hi])
                else:
                    nc.vector.tensor_copy(ot_bf, acc[:, lo:hi])
                rcols = small.tile([128, 4], F32, tag="rcols",
                                   name=f"rc{c}_{lo}")
                nb = len(ibs)
                if last:
                    # PE transpose of each [65,128] block; sumexp lands in
                    # column 64 of each 68-wide slot (4-byte aligned)
                    tp = psOt.tile([128, 68 * nb], BF16, tag="acc",
                                   name=f"tp{c}_{lo}")

                    def tp_mm():
                        for k in range(nb):
                            nc.tensor.transpose(
                                tp[:, k * 68:k * 68 + 65],
                                ot_bf[:, k * 128:(k + 1) * 128],
                                ident_sb[0:65, 0:65],
                            )
                    pending.append(tp_mm)

                    def recip():
                        nc.vector.reciprocal(
                            rcols[:, 0:nb],
                            tp.rearrange("p (b w) -> p b w", w=68)[:, 0:nb, 64])
                    pending.append(recip)
                else:
                    # sumexp row -> per-partition columns: tiny transposing
                    # DMAs spread across queues so they run in parallel
                    se_bf = small.tile([128, 4], BF16, tag="se_bf",
                                       name=f"se{c}_{lo}")
                    for k, ib in enumerate(ibs):
                        q = (nc.sync, nc.gpsimd)[
                            0 if c >= NCH - 2 else (findex[c] + k) % 2]
                        q.dma_start(
                            out=se_bf[:, k:k + 1],
                            in_=ot_bf[64:65, ib * 128 - lo:(ib + 1) * 128 - lo],
                        )

                    def recip():
                        nc.vector.reciprocal(rcols[:, 0:nb], se_bf[:, 0:nb])
                    pending.append(recip)
                pos = {}

                def po_mm(k, ib):
                    # independent of rcols — keeps the PE busy while the
                    # sumexp reciprocal chain resolves. The final part uses
                    # the accumulator banks (free by then) instead of psFil.
                    def go():
                        pool, tg = (psOt, "acc") if last else (psFil, "bank")
                        po = pool.tile([128, 512], F32, tag=tg,
                                       name=f"po{c}_{ib}")
                        pos[k] = po
                        nc.tensor.matmul(
                            po,
                            lhsT=ot_bf[0:64, ib * 128 - lo:(ib + 1) * 128 - lo],
                            rhs=wo_sb,
                            start=True,
                            stop=True,
                        )
                    return go

                def po_scale(k, ib):
                    def go():
                        ob = outp.tile([128, 512], BF16, tag="ob")
                        if last and k == 0:
                            nc.scalar.mul(ob, pos[k], rcols[:, k:k + 1])
                        elif last:
                            nc.vector.tensor_scalar_mul(
                                ob, pos[k], rcols[:, k:k + 1])
                        else:
                            evac_engine().tensor_scalar_mul(
                                ob, pos[k], rcols[:, k:k + 1])
                        oq = (nc.sync, nc.gpsimd)[
                            0 if c >= NCH - 2 else (findex[c] + k) % 2]
                        oq.dma_start(
                            out=out_d[c * 512 + ib * 128:
                                      c * 512 + (ib + 1) * 128, :],
                            in_=ob,
                        )
                    return go

                prev = None
                for k, ib in enumerate(ibs):
                    pending.append(po_mm(k, ib))
                    if prev is not None:
                        pending.append(po_scale(*prev))
                    prev = (k, ib)
                pending.append(po_scale(*prev))

            findex[c] += 1
            pending.append(evac)

        with tc.tile_pool(name="psB", bufs=2, space="PSUM") as psB, \
             tc.tile_pool(name="psFil", bufs=2, space="PSUM") as psFil, \
             tc.tile_pool(name="psOt", bufs=2, space="PSUM") as psOt:
            # filler work: remaining Q/K projections, V projections + V'
            # tiles. EMISSION ORDER IS LOGICAL ORDER in the tile framework:
            # jt0's score group over cols needing Q st_k must come after the
            # Q st_k filler (uniform 2-per-group drain satisfies this).
            for st in range(2, NCH):
                pending.append(filler_proj(wq_sb, 128, qt, st))
            for st in range(1, NCH):
                pending.append(filler_proj(wk_sb, 128, kt, st))
            for j2 in range(NJT):
                pending.append(vp_direct(j2))
            N_HEAD = len(pending)  # closures that still read the x^T tiles
            gdone = 0
            pt_blk, blk_off = None, 0
            for jt in range(NJT):
                i0 = jt * 128           # diagonal start
                # P^T tiles live in 4 merged block allocations (8 jt each):
                # pool-slot teardown costs ~4 semaphore ops per slot per
                # engine at kernel exit, so fewer slots = shorter epilogue
                if jt % 8 == 0:
                    blk_w = sum(S - 128 * j for j in range(jt, jt + 8))
                    pt_blk = pt_pool.tile([128, blk_w], BF16,
                                          tag=f"ptb{jt // 8}")
                    blk_off = 0
                pt = pt_blk[:, blk_off:blk_off + (S - i0)]
                blk_off += S - i0
                pts.append(pt)
                ktile = kt[:, jt * 128:(jt + 1) * 128]
                for gi, (g0, ghi) in enumerate(groups(jt)):
                    w = ghi - g0
                    ps = psB.tile([128, 1024], F32, tag="psB")
                    for lo in range(g0, ghi, 512):
                        hi = min(lo + 512, ghi)
                        nc.tensor.matmul(
                            ps[:, lo - g0:hi - g0],
                            lhsT=ktile,
                            rhs=qt[:, lo:hi],
                            start=True,
                            stop=True,
                        )
                    nc.scalar.activation(
                        pt[:, g0 - i0:ghi - i0],
                        ps[:, 0:w],
                        Exp,
                        scale=0.0625,  # 1/sqrt(64) / 2 (duplicated contraction)
                    )
                    gdone += 1
                    # pull PE filler work at a rate that empties the
                    # queue just as the exp stream ends
                    left = max(1, TOT_GROUPS - 6 - gdone)
                    drain(max(2, -(-len(pending) // left)))
                # causal mask on the diagonal 128x128 block (SBUF->SBUF, so
                # it can run on GpSimd — keeps the DVE queue short for the
                # PSUM evacuations that gate psFil slots)
                nc.vector.tensor_mul(pt[:, 0:128], pt[:, 0:128], mask_sb)
                c = jt // 4
                if jt % 4 == 1:
                    top_up(c, jt + 1)
                    if jt == NJT - 3 and EARLY_FIN:
                        # chunk 7 rows 3584-3840 are complete — finalize
                        # them under the remaining exp stream
                        enqueue_fin(c, 0, 256, last=False)
                elif jt % 4 == 2:
                    # rows [c*512, c*512+256) only need j2 <= 4c+1: finalize
                    # the first half early to spread evac/Wo/DMA load.
                    # (Reads the acc mid-accumulation-group on untouched
                    # columns — fine on HW, rejected by CoreSim; EARLY_FIN
                    # is turned off for sim validation runs.)
                    top_up(c, jt + 1)
                    if EARLY_FIN:
                        if jt == NJT - 2:
                            enqueue_fin(c, 256, 384, last=False)
                        else:
                            enqueue_fin(c, 0, 256, last=False)
                elif jt % 4 == 3:
                    top_up(c, jt + 1)
                    if c + 1 < NCH:
                        if EARLY_FIN:
                            enqueue_fin(c, 256, 512, last=False)
                        else:
                            enqueue_fin(c, 0, 512, last=False)
                        # pre-enqueue the next chunk's already-available
                        # contributions so its O^T work spreads out early
                        top_up(c + 1, jt + 1)
                    else:
                        if EARLY_FIN:
                            enqueue_fin(c, 384, 512, last=True)
                        else:
                            enqueue_fin(c, 0, 512, last=True)
                if jt == 19:
                    # open the last chunk extra-early (its accumulator takes
                    # the PSUM slot acc4 just vacated) so its O^T matmuls
                    # hide under the remaining exp stream instead of
                    # trailing it
                    top_up(NCH - 1, jt + 1)
                if jt == 9:
                    # finish everything that reads x^T, then free those tiles
                    # before the P^T pool reaches peak size; open chunk 3
                    # early (its slot was vacated back at jt=7) so the
                    # filler queue isn't dry right after this burst
                    drain(max(0, N_HEAD - state["done"]))
                    xtp_ctx.close()
                    top_up(3, jt + 1)
            while pending:
                drain(8)


def _build():
    if "nc" in _CACHE:
        return _CACHE["nc"]
    nc = bacc.Bacc("TRN2", target_bir_lowering=False, debug=False)
    with tile.TileContext(nc) as tc:
        _emit(nc, tc, None)
    nc.compile()
    _CACHE["nc"] = nc
    return nc


def build_in_maps(x, W_q, W_k, W_v, W_o):
    bf = ml_dtypes.bfloat16
    xT = np.ascontiguousarray(x.reshape(S, D_IN).T).astype(bf)
    mask = np.triu(np.ones((128, 128), np.float32)).astype(bf)
    ident = np.eye(128, dtype=np.float32).astype(bf)
    in_maps = []
    for h in range(H):
        wq2 = np.concatenate([W_q[h], W_q[h]], axis=1)  # [512, 128]
        wk2 = np.concatenate([W_k[h], W_k[h]], axis=1)
        in_maps.append({
            "xT": xT,
            "wq": np.ascontiguousarray(wq2).astype(bf),
            "wk": np.ascontiguousarray(wk2).astype(bf),
            "wv": np.ascontiguousarray(W_v[h]).astype(bf),
            "wo": np.ascontiguousarray(W_o[h]).astype(bf),
            "mask": mask,
            "ident": ident,
        })
    return in_maps


def kernel(x, W_q, W_k, W_v, W_o):
    nc = _build()
    in_maps = build_in_maps(x, W_q, W_k, W_v, W_o)
    res = run_bass_kernel_spmd(nc, in_maps, core_ids=list(range(H)))
    out = np.zeros((S, D_OUT), np.float32)
    for h in range(H):
        out += np.asarray(res.results[h]["out"], np.float32)
    return out[None]
